# revision 20
# baseline (speedup 1.0000x reference)
"""Trainium2 Bass kernel v3 for nn_Attention_70248485093458 (sliding-window attention).

8 cores x 2 query heads, 1 shared KV head. fp16 data paths, f32 psum/stats.

v3 changes over v2:
  - input DMA reorder (wv/wq-h0 first) + PE warmup dummies -> HAM warm early
  - host sends 4 rope rows [cos,sin,-sin,cos] (no device derivation)
  - one merged ACT staging copy per tile (v,k,q0,q1), sqs reads fp16 staging
  - both heads share one EXP / one affine_select per k-tile (merged psum)
  - all rs-chain SQRTs precede the first EXP -> 2 ACT table loads total
  - den psum DMA'd directly to DRAM; per-quad den row prefetch for the tail
  - attention A-parts j=0..6 moved after rs_block(3) at end of front
"""

import numpy as np

import concourse.bass as bass
import concourse.mybir as mybir
import concourse.tile as tile
from concourse import bacc
from concourse.bass import ts, ds
from concourse.masks import make_identity

T = 2048
DM = 256
DH = 256
WIN = 512
NCORES = 8
HPC = 2
P = 128
NT = 16
NB = 4
NQ = 4           # tq quads of 512
EPS = 1.1920929e-07
ROPE_BASE = 10000.0

F32 = mybir.dt.float32
F16 = mybir.dt.float16
F8 = mybir.dt.float8e4
DR = mybir.MatmulPerfMode.DoubleRow
MUL = mybir.AluOpType.mult
ADD = mybir.AluOpType.add
SUB = mybir.AluOpType.subtract
GE = mybir.AluOpType.is_ge
EXP = mybir.ActivationFunctionType.Exp
SQRT = mybir.ActivationFunctionType.Sqrt
COPY = mybir.ActivationFunctionType.Copy
SQUARE = mybir.ActivationFunctionType.Square


def _band(j):
    return min(WIN + P, T - P * j)


def build_program(apply_wprod: bool):
    nc = bacc.Bacc(None, target_bir_lowering=False)
    with tile.TileContext(nc) as tc, \
         tc.tile_pool(name="dram", bufs=1, space="DRAM") as dram, \
         tc.tile_pool(name="pp", bufs=1) as pp:
        x_d = dram.tile([P, 2, T], F16, kind="ExternalInput", name="xh", uniquify=False)
        wq_d = dram.tile([P, 2, 2, DH], F16, kind="ExternalInput", name="wqh", uniquify=False)
        wk_d = dram.tile([P, 2, DH], F16, kind="ExternalInput", name="wkh", uniquify=False)
        wv_d = dram.tile([P, 2, DH], F16, kind="ExternalInput", name="wvh", uniquify=False)
        wo_d = dram.tile([P, 4, DM], F16, kind="ExternalInput", name="woh", uniquify=False)
        # rope tables: host sends all 4 rows [cos, sin, -sin, cos]
        cs_d = dram.tile([P, 4, T], F16, kind="ExternalInput", name="csh", uniquify=False)
        wprod_d = None
        if apply_wprod:
            wprod_d = dram.tile([P, 2], F32, kind="ExternalInput", name="wprod", uniquify=False)
        o_d = dram.tile([P, NT, DM], F16, kind="ExternalOutput", name="o", uniquify=False)
        den_d = dram.tile([NQ, 2, 512], F32, kind="Internal", name="denq")

        # ---- persistent SBUF ----
        x_sb = pp.tile([P, 2, T], F16)
        wq_sb = pp.tile([P, 2, 2, DH], F16)
        wk_sb = pp.tile([P, 2, DH], F16)
        wv_sb = pp.tile([P, 2, DH], F16)
        wo_sb = pp.tile([P, 4, DM], F16)
        cs_sb = pp.tile([P, 4, T], F16)
        # staging: [.., 0, :]=v  [.., 1, :]=k  [.., 2+h, :]=q_h
        st_sb = pp.tile([P, NT, 4, DH], F16)
        # roped, normalized, transposed q/k: [p, head, comp, t] / [p, comp, t]
        # fp8e4: scores run as single DoubleRow matmuls (d=256 contraction)
        qTr = pp.tile([P, 2, 2, T], F8)
        kTr = pp.tile([P, 2, T], F8)
        # ssq -> rs in place: rows q0, q1, k
        ss = pp.tile([P, 3, NT], F32)
        attoT = pp.tile([P, 2, 2, T], F16)
        den_sb = pp.tile([1, 2, T], F32)
        rden = pp.tile([P, 2, NT], F32)
        den_rows = pp.tile([2 * NT, P], F32)   # rows (q, h, u)
        o_sb = pp.tile([P, NT, DM], F16)
        ones_h = pp.tile([P, 1], F16)
        ident = pp.tile([P, P], F16)
        ident32 = pp.tile([2 * NT, 2 * NT], F32)
        wprod_sb = pp.tile([P, 2], F32) if apply_wprod else None

        atp = pp.tile([P, 2, 7, 640], F16)
        negb = pp.tile([P, 1], F32)
        wscr = pp.tile([P, 512], F16)
        sqscr = pp.tile([P, 1], F32)
        nc.vector.memset(ones_h[:], 1.0)
        # exp offset: |s·rs_k| <= 16 hard (Cauchy-Schwarz on unit-rms vectors), so
        # exp(s-7) <= e^9 = 8103 fits fp16; typical s~N(0,1) keeps weights normal.
        nc.vector.memset(negb[:], -7.0)
        nc.vector.memset(wscr[:], 0.0)
        nc.vector.memset(sqscr[:], 1.0)
        # preload the sqrt ACT table during the input-DMA window; everything up
        # to the first EXP (copies, squares, sqrts) lives in the sqrt set.
        nc.scalar.activation(sqscr[:], sqscr[:], SQRT)
        make_identity(nc, ident[:])
        make_identity(nc, ident32[:])

        # ---- input DMAs ---- (minimal first-tile set leads each queue)
        nc.sync.dma_start(out=x_sb[:, :, 0:256], in_=x_d[:, :, 0:256])
        nc.sync.dma_start(out=x_sb[:, :, 256:512], in_=x_d[:, :, 256:512])
        nc.scalar.dma_start(out=wv_sb[:], in_=wv_d[:])
        nc.scalar.dma_start(out=wq_sb[:, :, 0, :], in_=wq_d[:, :, 0, :])
        nc.scalar.dma_start(out=wk_sb[:], in_=wk_d[:])
        nc.scalar.dma_start(out=wq_sb[:, :, 1, :], in_=wq_d[:, :, 1, :])
        for g in range(1, 4):
            nc.sync.dma_start(out=x_sb[:, :, ts(g, 512)], in_=x_d[:, :, ts(g, 512)])
        nc.scalar.dma_start(out=cs_sb[:, :, 0:512], in_=cs_d[:, :, 0:512])
        for g in range(1, 4):
            nc.scalar.dma_start(out=cs_sb[:, :, ts(g, 512)], in_=cs_d[:, :, ts(g, 512)])
        nc.sync.dma_start(out=wo_sb[:], in_=wo_d[:])
        if apply_wprod:
            nc.sync.dma_start(out=wprod_sb[:], in_=wprod_d[:])

        # =================== front ===================
        fp_cm = tc.tile_pool(name="fwork", bufs=1)
        fps_cm = tc.tile_pool(name="fpsum", bufs=1, space="PSUM")
        fw = fp_cm.__enter__()
        fps = fps_cm.__enter__()

        # PE warmup: ~12 dummy matmuls so HAM un-throttles before projections.
        # Shares a pp slot (released before the second projection needs it).
        warm_ps = fps.tile([1, 512], F32, tag="pp", bufs=2, name="warm")
        for _ in range(12):
            nc.tensor.matmul(warm_ps[:], ones_h[:], wscr[:], start=True, stop=True)

        attn = {}
        for _h in range(2):
            for _j in range(7):
                attn[(_h, _j)] = atp[:, _h, _j, :]
        pp_t = {}   # per-tile proj psum

        def proj_tile(n):
            # layout: v@bank0, k@bank0+1KB, q0@bank1, q1@bank1+1KB. Open
            # accumulation groups must not interleave within a bank, so pair
            # (v,q0) then (k,q1) — each pair hits two different banks.
            pps = fps.tile([P, 4, DH], F32, tag="pp", bufs=2, name=f"pp{n}")
            pp_t[n] = pps
            for cc in range(2):
                st = x_sb[:, cc, ts(n, P)]
                nc.tensor.matmul(pps[:, 0, :], st, wv_sb[:, cc, :],
                                 start=(cc == 0), stop=(cc == 1))
                nc.tensor.matmul(pps[:, 2, :], st, wq_sb[:, cc, 0, :],
                                 start=(cc == 0), stop=(cc == 1))
            for cc in range(2):
                st = x_sb[:, cc, ts(n, P)]
                nc.tensor.matmul(pps[:, 1, :], st, wk_sb[:, cc, :],
                                 start=(cc == 0), stop=(cc == 1))
                nc.tensor.matmul(pps[:, 3, :], st, wq_sb[:, cc, 1, :],
                                 start=(cc == 0), stop=(cc == 1))

        def ssq_copy_tile(n):
            pps = pp_t[n]
            # one merged staging copy: [v|k|q0|q1] psum -> st_sb fp16
            nc.scalar.activation(st_sb[:, n, :, :], pps[:], COPY)
            # ssq q1 on DVE (STT square + accum); q0/k on Scalar — balance
            sq0 = fw.tile([P, DH], F16, tag="sq0", bufs=4, name=f"sq0_{n}")
            nc.vector.scalar_tensor_tensor(
                out=sq0[:], in0=st_sb[:, n, 3, :], scalar=1.0,
                in1=st_sb[:, n, 3, :],
                op0=MUL, op1=MUL, accum_out=ss[:, 1, n : n + 1],
            )
            sqs = fw.tile([P, DH], F16, tag="sqs", bufs=4, name=f"sqs{n}")
            nc.scalar.activation(sqs[:], st_sb[:, n, 2, :], SQUARE,
                                 accum_out=ss[:, 0, n : n + 1])
            nc.scalar.activation(sqs[:], st_sb[:, n, 1, :], SQUARE,
                                 accum_out=ss[:, 2, n : n + 1])

        def rs_block(b):
            # one merged chain: rows q0,q1,k all use 1/sqrt(ssq + DH*EPS)
            # = 1/(16*rms).  q rows carry the extra 1/16 = score scale; k row
            # gets plain 1/rms (multiply by 16).
            cs3 = (slice(None), slice(0, 3), ds(4 * b, 4))
            nc.vector.tensor_scalar(out=ss[cs3], in0=ss[cs3],
                                    scalar1=1.0, scalar2=DH * EPS, op0=MUL, op1=ADD)
            nc.scalar.activation(ss[cs3], ss[cs3], SQRT)
            nc.vector.reciprocal(ss[cs3], ss[cs3])
            nc.vector.tensor_scalar(out=ss[:, 2, ds(4 * b, 4)],
                                    in0=ss[:, 2, ds(4 * b, 4)],
                                    scalar1=16.0, scalar2=0.0, op0=MUL, op1=ADD)

        def scale_tile(n):
            for h in range(2):
                nc.vector.tensor_scalar(out=st_sb[:, n, 2 + h, :],
                                        in0=st_sb[:, n, 2 + h, :],
                                        scalar1=ss[:, h, n : n + 1], scalar2=0.0,
                                        op0=MUL, op1=ADD)

        tr_b = {}

        def transpose_block(b):
            trp = fps.tile([P, 6, 4, P], F16, tag="tr", bufs=1, name=f"tr{b}")
            tr_b[b] = trp
            for u in range(4):
                n = 4 * b + u
                for t3 in range(3):
                    src = st_sb[:, n, 1, :] if t3 == 2 else st_sb[:, n, 2 + t3, :]
                    for c in range(2):
                        nc.tensor.transpose(
                            trp[:, 2 * t3 + c, u, :], src[:, ts(c, P)], ident[:]
                        )

        def rope_block(b):
            # merged-mul rope: u_e = [e|e] * [cos|sin], u_o = [o|o] * [-sin|cos],
            # [r_e|r_o] = u_e + u_o.  muls DVE (psum reads), add split Pool/DVE.
            trp = tr_b[b]
            sp = ts(b, 512)
            for t3 in range(3):
                if t3 == 2:
                    outro = kTr[:, :, sp]
                else:
                    outro = qTr[:, t3, :, sp]
                # one [P,2,2,512] mul: (te,to) x ([cos,sin],[-sin,cos])
                u4 = fw.tile([P, 2, 2, 512], F16, tag=f"u{t3}", bufs=2,
                             name=f"u4_{t3}{b}")
                t2 = trp[:, 2 * t3 : 2 * t3 + 2, :, :] \
                    .rearrange("p a b c -> p a (b c)") \
                    .rearrange("p a (x d) -> p a x d", x=1) \
                    .broadcast_to((P, 2, 2, 512))
                c2 = cs_sb[:, :, sp].rearrange("p (a b) t -> p a b t", a=2)
                nc.vector.tensor_mul(u4[:], t2, c2)
                # split the add across Pool (e) and DVE (o) — Pool is ~2x slower
                nc.gpsimd.tensor_add(outro[:, 0, :], u4[:, 0, 0, :], u4[:, 1, 0, :])
                nc.vector.tensor_add(outro[:, 1, :], u4[:, 0, 1, :], u4[:, 1, 1, :])
            if apply_wprod:
                for c in range(2):
                    nc.vector.tensor_scalar(
                        out=kTr[:, c, sp], in0=kTr[:, c, sp],
                        scalar1=wprod_sb[:, c : c + 1], scalar2=0.0, op0=MUL, op1=ADD)

        def a_part(pj):
            sAp = fps.tile([P, 2, 512], F32, tag="pp", bufs=2, name=f"sAp{pj}")
            for ph in range(2):
                nc.tensor.matmul(
                    sAp[:, ph, :], kTr[:, :, ts(pj, P)],
                    qTr[:, ph, :, ds(P * pj, 512)],
                    start=True, stop=True, perf_mode=DR)
            nc.scalar.activation(atp[:, :, pj, 0:512], sAp[:], EXP,
                                 scale=ss[:, 2, pj : pj + 1], bias=negb[:])
            nc.gpsimd.affine_select(
                out=atp[:, :, pj, 0:P], in_=atp[:, :, pj, 0:P], compare_op=GE,
                fill=0.0, base=0, pattern=[[0, 2], [1, P]],
                channel_multiplier=-1)

        # front schedule: transposes of block b-1 ride behind projections of b;
        # attention A-parts interleave once their qTr blocks (j+4)//4 are roped.
        for b in range(NB):
            for u in range(4):
                proj_tile(4 * b + u)
                ssq_copy_tile(4 * b + u)
            rs_block(b)
            for u in range(4):
                scale_tile(4 * b + u)
            if b >= 1:
                transpose_block(b - 1)
                rope_block(b - 1)
            if b == 2:
                for pj in range(4):
                    a_part(pj)
            if b == 3:
                for pj in range(4, 7):
                    a_part(pj)
        transpose_block(NB - 1)
        rope_block(NB - 1)

        fps_cm.__exit__(None, None, None)
        fp_cm.__exit__(None, None, None)

        # =================== attention ===================
        ap_cm = tc.tile_pool(name="awork", bufs=1)
        aps_cm = tc.tile_pool(name="apsum", bufs=1, space="PSUM")
        aw = ap_cm.__enter__()
        aps = aps_cm.__enter__()

        def do_quad(q):
            js = list(range(max(0, 4 * q - 4), min(NT, 4 * q + 4)))
            js.remove(4 * q)
            js.insert(0, 4 * q)  # full-width segment first (uniform has_written)
            # heads run serially so pv needs only one psum slot; den (PE) and
            # the attoT copy (DVE) of head h fill the gap before head h+1.
            for h in range(2):
                a2 = aps.tile([P, 2, 512], F32, tag="pv", bufs=1, name=f"aq{h}_{q}")
                for ji, jj in enumerate(js):
                    wj = _band(jj)
                    lo = max(0, 512 * q - P * jj)
                    hi = min(wj, 512 * q + 512 - P * jj)
                    po = P * jj + lo - 512 * q
                    for c in range(2):
                        nc.tensor.matmul(
                            a2[:, c, po : po + hi - lo],
                            st_sb[:, jj, 0, ts(c, P)], attn[(h, jj)][:, lo:hi],
                            start=(ji == 0), stop=(ji == len(js) - 1))
                d_ps = aps.tile([1, 512], F32, tag="dq", bufs=1, name=f"dq{h}_{q}")
                for ji, jj in enumerate(js):
                    wj = _band(jj)
                    lo = max(0, 512 * q - P * jj)
                    hi = min(wj, 512 * q + 512 - P * jj)
                    po = P * jj + lo - 512 * q
                    nc.tensor.matmul(
                        d_ps[:, po : po + hi - lo], ones_h[:],
                        attn[(h, jj)][:, lo:hi],
                        start=(ji == 0), stop=(ji == len(js) - 1))
                nc.vector.tensor_copy(attoT[:, h, :, ts(q, 512)], a2[:])
                nc.scalar.activation(den_sb[:, h, ts(q, 512)], d_ps[:], COPY)
                # per-quad den bounce to DRAM while the sync queue is idle
                nc.sync.dma_start(out=den_d[q, h, :],
                                  in_=den_sb[:, h, ts(q, 512)])
                # prefetch den rows for the tail transpose
                nc.sync.dma_start(
                    out=den_rows[ds(8 * q + 4 * h, 4), :],
                    in_=den_d[q, h, :].rearrange("(n u) -> n u", u=P))

        for j in range(NT):
            w = _band(j)
            wA = min(w, 512)
            if j < 7:
                # A-parts were computed during the front phase; finish B-parts
                sBp = aps.tile([P, 2, P], F32, tag="sB", bufs=1, name=f"sBp_{j}")
                for h in range(2):
                    nc.tensor.matmul(
                        sBp[:, h, :], kTr[:, :, ts(j, P)],
                        qTr[:, h, :, ds(P * j + 512, P)],
                        start=True, stop=True, perf_mode=DR)
                nc.scalar.activation(atp[:, :, j, 512:640], sBp[:], EXP,
                                     scale=ss[:, 2, j : j + 1], bias=negb[:])
                nc.gpsimd.affine_select(
                    out=atp[:, :, j, WIN : WIN + P], in_=atp[:, :, j, WIN : WIN + P],
                    compare_op=GE, fill=0.0,
                    base=0, pattern=[[0, 2], [-1, P]], channel_multiplier=1)
                if j % 4 == 3:
                    do_quad(j // 4)
                continue
            # heads share one psum/EXP; per c each kTr stationary loads once
            at2 = aw.tile([P, 2, 640], F16, tag="at", bufs=9, name=f"at_{j}")
            sA2 = aps.tile([P, 2, 512], F32, tag="sA", bufs=2, name=f"sA_{j}")
            for h in range(2):
                attn[(h, j)] = at2[:, h, :]
            sB = (aps.tile([P, 2, P], F32, tag="sB", bufs=1, name=f"sB_{j}")
                  if w == 640 else None)
            for h in range(2):
                nc.tensor.matmul(
                    sA2[:, h, 0:wA], kTr[:, :, ts(j, P)],
                    qTr[:, h, :, ds(P * j, wA)],
                    start=True, stop=True, perf_mode=DR)
                if w == 640:
                    nc.tensor.matmul(
                        sB[:, h, :], kTr[:, :, ts(j, P)],
                        qTr[:, h, :, ds(P * j + 512, P)],
                        start=True, stop=True, perf_mode=DR)
            nc.scalar.activation(at2[:, :, 0:wA], sA2[:, :, 0:wA], EXP,
                                 scale=ss[:, 2, j : j + 1], bias=negb[:])
            if w == 640:
                nc.scalar.activation(at2[:, :, 512:640], sB[:], EXP,
                                     scale=ss[:, 2, j : j + 1], bias=negb[:])
            # causal mask on diagonal P cols: keep tq_local >= tk
            nc.gpsimd.affine_select(
                out=at2[:, :, 0:P], in_=at2[:, :, 0:P], compare_op=GE, fill=0.0,
                base=0, pattern=[[0, 2], [1, P]], channel_multiplier=-1)
            if w == 640:
                # window mask on last P cols: keep tk >= tq_local-512
                nc.gpsimd.affine_select(
                    out=at2[:, :, WIN : WIN + P], in_=at2[:, :, WIN : WIN + P],
                    compare_op=GE, fill=0.0,
                    base=0, pattern=[[0, 2], [-1, P]], channel_multiplier=1)

            if j % 4 == 3:
                do_quad(j // 4)

        aps_cm.__exit__(None, None, None)
        ap_cm.__exit__(None, None, None)

        # =================== tail: out projection ===================
        with tc.tile_pool(name="tpsum", bufs=1, space="PSUM") as tps:
            dtr = tps.tile([P, 2 * NT], F32, tag="dtr", bufs=1, name="dtr")
            nc.tensor.transpose(dtr[:], den_rows[:], ident32[:])
            nc.vector.reciprocal(
                rden[:].rearrange("p h (q n) -> p h q n", n=4),
                dtr[:].rearrange("p (q h n) -> p h q n", h=2, n=4))
            for n in range(NT):
                o_ps = tps.tile([P, DM], F32, tag="op0", bufs=3, name=f"op0_{n}")
                for c in range(2):
                    nc.tensor.matmul(o_ps[:], attoT[:, 0, c, ts(n, P)],
                                     wo_sb[:, c, :], start=(c == 0), stop=(c == 1))
                nc.scalar.activation(o_sb[:, n, :], o_ps[:], COPY,
                                     scale=rden[:, 0, n : n + 1])
                o_ps2 = tps.tile([P, DM], F32, tag="op1", bufs=3, name=f"op1_{n}")
                for c in range(2):
                    nc.tensor.matmul(o_ps2[:], attoT[:, 1, c, ts(n, P)],
                                     wo_sb[:, 2 + c, :], start=(c == 0), stop=(c == 1))
                nc.vector.scalar_tensor_tensor(
                    out=o_sb[:, n, :], in0=o_ps2[:], scalar=rden[:, 1, n : n + 1],
                    in1=o_sb[:, n, :], op0=MUL, op1=ADD)
                if n % 2 == 1:
                    nc.sync.dma_start(out=o_d[:, n - 1 : n + 1, :],
                                      in_=o_sb[:, n - 1 : n + 1, :])
    nc.compile()
    return nc


# ======================= host side =======================

_PROGRAMS = {}


def _get_program(apply_wprod: bool):
    key = bool(apply_wprod)
    if key not in _PROGRAMS:
        _PROGRAMS[key] = build_program(key)
    return _PROGRAMS[key]


_DEINT = np.concatenate([np.arange(0, DH, 2), np.arange(1, DH, 2)])


def _rope_tables():
    freqs = ROPE_BASE ** (-2.0 * np.arange(DH // 2, dtype=np.float64) / DH)
    theta = np.arange(T, dtype=np.float64)[None, :] * freqs[:, None]
    return np.cos(theta), np.sin(theta)


def _prep_inputs(x, wq, wkv, wo, q_norm_w, k_norm_w):
    x2 = np.asarray(x, dtype=np.float32).reshape(T, DM)
    wq = np.asarray(wq, dtype=np.float32)
    wkv = np.asarray(wkv, dtype=np.float32)
    wo = np.asarray(wo, dtype=np.float32)
    wk = wkv[:DH]
    wv = wkv[DH:]
    cos, sin = _rope_tables()

    # x transposed: xh[p, cc, t] = x[t, 128cc+p]
    xh = np.ascontiguousarray(
        x2.T.reshape(2, P, T).transpose(1, 0, 2).astype(np.float16))
    # cs[p, row, t] rows [cos, sin, -sin, cos]
    csh = np.ascontiguousarray(
        np.stack([cos, sin, -sin, cos], axis=1).astype(np.float16))
    # k/v: [p, cc, dh_out]
    wkT = wk[_DEINT].T.astype(np.float16)          # [dm, dh]
    wvT = wv.T.astype(np.float16)
    wkh = np.ascontiguousarray(wkT.reshape(2, P, DH).transpose(1, 0, 2))
    wvh = np.ascontiguousarray(wvT.reshape(2, P, DH).transpose(1, 0, 2))

    wprod = (np.asarray(q_norm_w, np.float32) * np.asarray(k_norm_w, np.float32))
    apply_wprod = not np.allclose(wprod, 1.0)
    wprod_de = wprod[_DEINT]
    wprod_cols = np.ascontiguousarray(
        np.stack([wprod_de[:P], wprod_de[P:]], axis=1).astype(np.float32))

    in_maps = []
    for c in range(NCORES):
        heads = [HPC * c + i for i in range(HPC)]
        wqh = np.empty((P, 2, 2, DH), dtype=np.float16)   # [p, cc, h, dh]
        for hl, h in enumerate(heads):
            wqT = wq[DH * h : DH * (h + 1)][_DEINT].T.astype(np.float16)  # [dm, dh]
            wqh[:, :, hl, :] = wqT.reshape(2, P, DH).transpose(1, 0, 2)
        woh = np.empty((P, 4, DM), dtype=np.float16)      # [p, 2h+c, dm]
        for hl, h in enumerate(heads):
            woT = wo[:, DH * h : DH * (h + 1)].T.astype(np.float16)  # [dh, dm]
            woh[:, 2 * hl, :] = woT[0:P]
            woh[:, 2 * hl + 1, :] = woT[P : 2 * P]
        m = {
            "xh": xh, "wqh": np.ascontiguousarray(wqh), "wkh": wkh, "wvh": wvh,
            "woh": np.ascontiguousarray(woh), "csh": csh,
        }
        if apply_wprod:
            m["wprod"] = wprod_cols
        in_maps.append(m)
    return in_maps, apply_wprod


def _run(inputs, trace=False, trace_kwargs=None):
    from concourse.bass_utils import run_bass_kernel_spmd

    in_maps, apply_wprod = _prep_inputs(**inputs)
    nc = _get_program(apply_wprod)
    res = run_bass_kernel_spmd(
        nc, in_maps, list(range(NCORES)), trace=trace, **(trace_kwargs or {})
    )
    out = np.zeros((T, DM), dtype=np.float32)
    for c in range(NCORES):
        oc = np.asarray(res.results[c]["o"], dtype=np.float32)  # [p, n, dm]
        out += oc.transpose(1, 0, 2).reshape(T, DM)
    return out.reshape(1, T, DM), res


def kernel(**inputs):
    out, _ = _run(inputs, trace=False)
    return out


# revision 25
# speedup vs baseline: 1.0499x; 1.0499x over previous
"""Trainium2 Bass kernel v3 for nn_Attention_70248485093458 (sliding-window attention).

8 cores x 2 query heads, 1 shared KV head. fp16 data paths, f32 psum/stats.

v3 changes over v2:
  - input DMA reorder (wv/wq-h0 first) + PE warmup dummies -> HAM warm early
  - host sends 4 rope rows [cos,sin,-sin,cos] (no device derivation)
  - one merged ACT staging copy per tile (v,k,q0,q1), sqs reads fp16 staging
  - both heads share one EXP / one affine_select per k-tile (merged psum)
  - all rs-chain SQRTs precede the first EXP -> 2 ACT table loads total
  - den psum DMA'd directly to DRAM; per-quad den row prefetch for the tail
  - attention A-parts j=0..6 moved after rs_block(3) at end of front
"""

import numpy as np

import concourse.bass as bass
import concourse.mybir as mybir
import concourse.tile as tile
from concourse import bacc
from concourse.bass import ts, ds
from concourse.masks import make_identity

T = 2048
DM = 256
DH = 256
WIN = 512
NCORES = 8
HPC = 2
P = 128
NT = 16
NB = 4
NQ = 4           # tq quads of 512
EPS = 1.1920929e-07
ROPE_BASE = 10000.0

F32 = mybir.dt.float32
F16 = mybir.dt.float16
F8 = mybir.dt.float8e4
DR = mybir.MatmulPerfMode.DoubleRow
MUL = mybir.AluOpType.mult
ADD = mybir.AluOpType.add
SUB = mybir.AluOpType.subtract
GE = mybir.AluOpType.is_ge
EXP = mybir.ActivationFunctionType.Exp
SQRT = mybir.ActivationFunctionType.Sqrt
COPY = mybir.ActivationFunctionType.Copy
SQUARE = mybir.ActivationFunctionType.Square


def _band(j):
    return min(WIN + P, T - P * j)


def build_program(apply_wprod: bool):
    nc = bacc.Bacc(None, target_bir_lowering=False)
    with tile.TileContext(nc) as tc, \
         tc.tile_pool(name="dram", bufs=1, space="DRAM") as dram, \
         tc.tile_pool(name="pp", bufs=1) as pp:
        x_d = dram.tile([P, 2, T], F16, kind="ExternalInput", name="xh", uniquify=False)
        wq_d = dram.tile([P, 2, 2, DH], F16, kind="ExternalInput", name="wqh", uniquify=False)
        wk_d = dram.tile([P, 2, DH], F16, kind="ExternalInput", name="wkh", uniquify=False)
        wv_d = dram.tile([P, 2, DH], F16, kind="ExternalInput", name="wvh", uniquify=False)
        wo_d = dram.tile([P, 4, DM], F16, kind="ExternalInput", name="woh", uniquify=False)
        # rope tables: host sends all 4 rows [cos, sin, -sin, cos]
        cs_d = dram.tile([P, 4, T], F16, kind="ExternalInput", name="csh", uniquify=False)
        wprod_d = None
        if apply_wprod:
            wprod_d = dram.tile([P, 2], F32, kind="ExternalInput", name="wprod", uniquify=False)
        o_d = dram.tile([P, NT, DM], F16, kind="ExternalOutput", name="o", uniquify=False)
        den_d = dram.tile([NQ, 2, 512], F32, kind="Internal", name="denq")

        # ---- persistent SBUF ----
        x_sb = pp.tile([P, 2, T], F16)
        wq_sb = pp.tile([P, 2, 2, DH], F16)
        wk_sb = pp.tile([P, 2, DH], F16)
        wv_sb = pp.tile([P, 2, DH], F16)
        wo_sb = pp.tile([P, 4, DM], F16)
        cs_sb = pp.tile([P, 4, T], F16)
        # staging: [.., 0, :]=v  [.., 1, :]=k  [.., 2+h, :]=q_h
        st_sb = pp.tile([P, NT, 4, DH], F16)
        # roped, normalized, transposed q/k: [p, head, comp, t] / [p, comp, t]
        # fp8e4: scores run as single DoubleRow matmuls (d=256 contraction)
        qTr = pp.tile([P, 2, 2, T], F8)
        kTr = pp.tile([P, 2, T], F8)
        # ssq -> rs in place: rows q0, q1, k
        ss = pp.tile([P, 3, NT], F32)
        attoT = pp.tile([P, 2, 2, T], F16)
        den_sb = pp.tile([1, 2, T], F32)
        rden = pp.tile([P, 2, NT], F32)
        den_rows = pp.tile([2 * NT, P], F32)   # rows (q, h, u)
        o_sb = pp.tile([P, NT, DM], F16)
        ones_h = pp.tile([P, 1], F16)
        ident = pp.tile([P, P], F16)
        ident32 = pp.tile([2 * NT, 2 * NT], F32)
        wprod_sb = pp.tile([P, 2], F32) if apply_wprod else None

        atp = pp.tile([P, 2, 7, 640], F16)
        negb = pp.tile([P, 1], F32)
        wscr = pp.tile([P, 512], F16)
        sqscr = pp.tile([P, 1], F32)
        nc.vector.memset(ones_h[:], 1.0)
        # exp offset: |s·rs_k| <= 16 hard (Cauchy-Schwarz on unit-rms vectors), so
        # exp(s-7) <= e^9 = 8103 fits fp16; typical s~N(0,1) keeps weights normal.
        nc.vector.memset(negb[:], -7.0)
        nc.vector.memset(wscr[:], 0.0)
        nc.vector.memset(sqscr[:], 1.0)
        # preload the sqrt ACT table during the input-DMA window; everything up
        # to the first EXP (copies, squares, sqrts) lives in the sqrt set.
        nc.scalar.activation(sqscr[:], sqscr[:], SQRT)
        make_identity(nc, ident[:])
        make_identity(nc, ident32[:])

        # ---- input DMAs ---- (minimal first-tile set leads each queue)
        nc.sync.dma_start(out=x_sb[:, :, 0:256], in_=x_d[:, :, 0:256])
        nc.sync.dma_start(out=x_sb[:, :, 256:512], in_=x_d[:, :, 256:512])
        nc.scalar.dma_start(out=wv_sb[:], in_=wv_d[:])
        nc.scalar.dma_start(out=wq_sb[:, :, 0, :], in_=wq_d[:, :, 0, :])
        nc.scalar.dma_start(out=wk_sb[:], in_=wk_d[:])
        nc.scalar.dma_start(out=wq_sb[:, :, 1, :], in_=wq_d[:, :, 1, :])
        for g in range(1, 4):
            nc.sync.dma_start(out=x_sb[:, :, ts(g, 512)], in_=x_d[:, :, ts(g, 512)])
        nc.scalar.dma_start(out=cs_sb[:, :, 0:512], in_=cs_d[:, :, 0:512])
        for g in range(1, 4):
            nc.scalar.dma_start(out=cs_sb[:, :, ts(g, 512)], in_=cs_d[:, :, ts(g, 512)])
        nc.sync.dma_start(out=wo_sb[:], in_=wo_d[:])
        if apply_wprod:
            nc.sync.dma_start(out=wprod_sb[:], in_=wprod_d[:])

        # =================== front ===================
        fp_cm = tc.tile_pool(name="fwork", bufs=1)
        fps_cm = tc.tile_pool(name="fpsum", bufs=1, space="PSUM")
        fw = fp_cm.__enter__()
        fps = fps_cm.__enter__()

        # PE warmup: ~12 dummy matmuls so HAM un-throttles before projections.
        # Shares a pp slot (released before the second projection needs it).
        warm_ps = fps.tile([1, 512], F32, tag="pp", bufs=2, name="warm")
        for _ in range(12):
            nc.tensor.matmul(warm_ps[:], ones_h[:], wscr[:], start=True, stop=True)

        attn = {}
        for _h in range(2):
            for _j in range(7):
                attn[(_h, _j)] = atp[:, _h, _j, :]
        pp_t = {}   # per-tile proj psum

        def proj_tile(n):
            # layout: v@bank0, k@bank0+1KB, q0@bank1, q1@bank1+1KB. Open
            # accumulation groups must not interleave within a bank, so pair
            # (v,q0) then (k,q1) — each pair hits two different banks.
            pps = fps.tile([P, 4, DH], F32, tag="pp", bufs=2, name=f"pp{n}")
            pp_t[n] = pps
            for cc in range(2):
                st = x_sb[:, cc, ts(n, P)]
                nc.tensor.matmul(pps[:, 0, :], st, wv_sb[:, cc, :],
                                 start=(cc == 0), stop=(cc == 1))
                nc.tensor.matmul(pps[:, 2, :], st, wq_sb[:, cc, 0, :],
                                 start=(cc == 0), stop=(cc == 1))
            for cc in range(2):
                st = x_sb[:, cc, ts(n, P)]
                nc.tensor.matmul(pps[:, 1, :], st, wk_sb[:, cc, :],
                                 start=(cc == 0), stop=(cc == 1))
                nc.tensor.matmul(pps[:, 3, :], st, wq_sb[:, cc, 1, :],
                                 start=(cc == 0), stop=(cc == 1))

        def ssq_copy_tile(n):
            pps = pp_t[n]
            # one merged staging copy: [v|k|q0|q1] psum -> st_sb fp16
            nc.scalar.activation(st_sb[:, n, :, :], pps[:], COPY)
            # ssq q0/q1 on DVE from fp16 staging (STT square + accum)
            sq0 = fw.tile([P, DH], F16, tag="sq0", bufs=4, name=f"sq0_{n}")
            for h in range(2):
                nc.vector.scalar_tensor_tensor(
                    out=sq0[:], in0=st_sb[:, n, 2 + h, :], scalar=1.0,
                    in1=st_sb[:, n, 2 + h, :],
                    op0=MUL, op1=MUL, accum_out=ss[:, h, n : n + 1],
                )
            # ssq k on Scalar (square w/ scratch out) from fp16 staging
            sqs = fw.tile([P, DH], F16, tag="sqs", bufs=4, name=f"sqs{n}")
            nc.scalar.activation(sqs[:], st_sb[:, n, 1, :], SQUARE,
                                 accum_out=ss[:, 2, n : n + 1])

        def rs_block(b):
            # one merged chain: rows q0,q1,k all use 1/sqrt(ssq + DH*EPS)
            # = 1/(16*rms).  q rows carry the extra 1/16 = score scale; k row
            # gets plain 1/rms (multiply by 16).
            cs3 = (slice(None), slice(0, 3), ds(4 * b, 4))
            nc.vector.tensor_scalar(out=ss[cs3], in0=ss[cs3],
                                    scalar1=1.0, scalar2=DH * EPS, op0=MUL, op1=ADD)
            nc.scalar.activation(ss[cs3], ss[cs3], SQRT)
            nc.vector.reciprocal(ss[cs3], ss[cs3])
            nc.vector.tensor_scalar(out=ss[:, 2, ds(4 * b, 4)],
                                    in0=ss[:, 2, ds(4 * b, 4)],
                                    scalar1=16.0, scalar2=0.0, op0=MUL, op1=ADD)

        def scale_tile(n):
            for h in range(2):
                nc.vector.tensor_scalar(out=st_sb[:, n, 2 + h, :],
                                        in0=st_sb[:, n, 2 + h, :],
                                        scalar1=ss[:, h, n : n + 1], scalar2=0.0,
                                        op0=MUL, op1=ADD)

        tr_b = {}

        def transpose_block(b):
            trp = fps.tile([P, 6, 4, P], F16, tag="tr", bufs=1, name=f"tr{b}")
            tr_b[b] = trp
            for u in range(4):
                n = 4 * b + u
                for t3 in range(3):
                    src = st_sb[:, n, 1, :] if t3 == 2 else st_sb[:, n, 2 + t3, :]
                    for c in range(2):
                        nc.tensor.transpose(
                            trp[:, 2 * t3 + c, u, :], src[:, ts(c, P)], ident[:]
                        )

        def rope_block(b):
            # merged-mul rope: u_e = [e|e] * [cos|sin], u_o = [o|o] * [-sin|cos],
            # [r_e|r_o] = u_e + u_o.  muls DVE (psum reads), add split Pool/DVE.
            trp = tr_b[b]
            sp = ts(b, 512)
            for t3 in range(3):
                if t3 == 2:
                    outro = kTr[:, :, sp]
                else:
                    outro = qTr[:, t3, :, sp]
                # one [P,2,2,512] mul: (te,to) x ([cos,sin],[-sin,cos])
                u4 = fw.tile([P, 2, 2, 512], F16, tag=f"u{t3}", bufs=2,
                             name=f"u4_{t3}{b}")
                t2 = trp[:, 2 * t3 : 2 * t3 + 2, :, :] \
                    .rearrange("p a b c -> p a (b c)") \
                    .rearrange("p a (x d) -> p a x d", x=1) \
                    .broadcast_to((P, 2, 2, 512))
                c2 = cs_sb[:, :, sp].rearrange("p (a b) t -> p a b t", a=2)
                nc.vector.tensor_mul(u4[:], t2, c2)
                # split the add across Pool (e) and DVE (o) — Pool is ~2x slower
                nc.gpsimd.tensor_add(outro[:, 0, :], u4[:, 0, 0, :], u4[:, 1, 0, :])
                nc.vector.tensor_add(outro[:, 1, :], u4[:, 0, 1, :], u4[:, 1, 1, :])
            if apply_wprod:
                for c in range(2):
                    nc.vector.tensor_scalar(
                        out=kTr[:, c, sp], in0=kTr[:, c, sp],
                        scalar1=wprod_sb[:, c : c + 1], scalar2=0.0, op0=MUL, op1=ADD)

        def a_part(pj):
            sAp = fps.tile([P, 2, 512], F32, tag="pp", bufs=2, name=f"sAp{pj}")
            for ph in range(2):
                nc.tensor.matmul(
                    sAp[:, ph, :], kTr[:, :, ts(pj, P)],
                    qTr[:, ph, :, ds(P * pj, 512)],
                    start=True, stop=True, perf_mode=DR)
            nc.scalar.activation(atp[:, :, pj, 0:512], sAp[:], EXP,
                                 scale=ss[:, 2, pj : pj + 1], bias=negb[:])
            nc.gpsimd.affine_select(
                out=atp[:, :, pj, 0:P], in_=atp[:, :, pj, 0:P], compare_op=GE,
                fill=0.0, base=0, pattern=[[0, 2], [1, P]],
                channel_multiplier=-1)

        # front schedule: transposes of block b-1 ride behind projections of b;
        # attention A-parts interleave once their qTr blocks (j+4)//4 are roped.
        for b in range(NB):
            for u in range(4):
                proj_tile(4 * b + u)
                ssq_copy_tile(4 * b + u)
            rs_block(b)
            for u in range(4):
                scale_tile(4 * b + u)
            if b >= 1:
                transpose_block(b - 1)
                rope_block(b - 1)
            if b == 3:
                # all rs_block SQRTs precede the first EXP (2 ACT table loads)
                for pj in range(7):
                    a_part(pj)
        transpose_block(NB - 1)
        rope_block(NB - 1)

        fps_cm.__exit__(None, None, None)
        fp_cm.__exit__(None, None, None)

        # =================== attention ===================
        ap_cm = tc.tile_pool(name="awork", bufs=1)
        aps_cm = tc.tile_pool(name="apsum", bufs=1, space="PSUM")
        aw = ap_cm.__enter__()
        aps = aps_cm.__enter__()

        def do_quad(q):
            js = list(range(max(0, 4 * q - 4), min(NT, 4 * q + 4)))
            js.remove(4 * q)
            js.insert(0, 4 * q)  # full-width segment first (uniform has_written)
            # both heads interleaved per segment: each v stationary loads once
            a2 = [aps.tile([P, 2, 512], F32, tag="pv", bufs=2, name=f"aq{h}_{q}")
                  for h in range(2)]
            for ji, jj in enumerate(js):
                wj = _band(jj)
                lo = max(0, 512 * q - P * jj)
                hi = min(wj, 512 * q + 512 - P * jj)
                po = P * jj + lo - 512 * q
                for c in range(2):
                    for h in range(2):
                        nc.tensor.matmul(
                            a2[h][:, c, po : po + hi - lo],
                            st_sb[:, jj, 0, ts(c, P)], attn[(h, jj)][:, lo:hi],
                            start=(ji == 0), stop=(ji == len(js) - 1))
            for h in range(2):
                d_ps = aps.tile([1, 512], F32, tag="dq", bufs=1, name=f"dq{h}_{q}")
                for ji, jj in enumerate(js):
                    wj = _band(jj)
                    lo = max(0, 512 * q - P * jj)
                    hi = min(wj, 512 * q + 512 - P * jj)
                    po = P * jj + lo - 512 * q
                    nc.tensor.matmul(
                        d_ps[:, po : po + hi - lo], ones_h[:],
                        attn[(h, jj)][:, lo:hi],
                        start=(ji == 0), stop=(ji == len(js) - 1))
                nc.vector.tensor_copy(attoT[:, h, :, ts(q, 512)], a2[h][:])
                nc.scalar.activation(den_sb[:, h, ts(q, 512)], d_ps[:], COPY)
                # per-quad den bounce to DRAM while the sync queue is idle
                nc.sync.dma_start(out=den_d[q, h, :],
                                  in_=den_sb[:, h, ts(q, 512)])
                # prefetch den rows for the tail transpose
                nc.sync.dma_start(
                    out=den_rows[ds(8 * q + 4 * h, 4), :],
                    in_=den_d[q, h, :].rearrange("(n u) -> n u", u=P))

        for j in range(NT):
            w = _band(j)
            wA = min(w, 512)
            if j < 7:
                # A-parts were computed during the front phase; finish B-parts
                sBp = aps.tile([P, 2, P], F32, tag="sB", bufs=1, name=f"sBp_{j}")
                for h in range(2):
                    nc.tensor.matmul(
                        sBp[:, h, :], kTr[:, :, ts(j, P)],
                        qTr[:, h, :, ds(P * j + 512, P)],
                        start=True, stop=True, perf_mode=DR)
                nc.scalar.activation(atp[:, :, j, 512:640], sBp[:], EXP,
                                     scale=ss[:, 2, j : j + 1], bias=negb[:])
                nc.gpsimd.affine_select(
                    out=atp[:, :, j, WIN : WIN + P], in_=atp[:, :, j, WIN : WIN + P],
                    compare_op=GE, fill=0.0,
                    base=0, pattern=[[0, 2], [-1, P]], channel_multiplier=1)
                if j % 4 == 3:
                    do_quad(j // 4)
                continue
            # heads share one psum/EXP; per c each kTr stationary loads once
            at2 = aw.tile([P, 2, 640], F16, tag="at", bufs=9, name=f"at_{j}")
            for h in range(2):
                attn[(h, j)] = at2[:, h, :]
            sB = (aps.tile([P, 2, P], F32, tag="sB", bufs=1, name=f"sB_{j}")
                  if w == 640 else None)
            for h in range(2):
                sAh = aps.tile([P, 512], F32, tag="sA", bufs=2, name=f"sA{h}_{j}")
                nc.tensor.matmul(
                    sAh[:, 0:wA], kTr[:, :, ts(j, P)],
                    qTr[:, h, :, ds(P * j, wA)],
                    start=True, stop=True, perf_mode=DR)
                if w == 640:
                    nc.tensor.matmul(
                        sB[:, h, :], kTr[:, :, ts(j, P)],
                        qTr[:, h, :, ds(P * j + 512, P)],
                        start=True, stop=True, perf_mode=DR)
                nc.scalar.activation(at2[:, h, 0:wA], sAh[:, 0:wA], EXP,
                                     scale=ss[:, 2, j : j + 1], bias=negb[:])
            if w == 640:
                nc.scalar.activation(at2[:, :, 512:640], sB[:], EXP,
                                     scale=ss[:, 2, j : j + 1], bias=negb[:])
            # causal mask on diagonal P cols: keep tq_local >= tk
            nc.gpsimd.affine_select(
                out=at2[:, :, 0:P], in_=at2[:, :, 0:P], compare_op=GE, fill=0.0,
                base=0, pattern=[[0, 2], [1, P]], channel_multiplier=-1)
            if w == 640:
                # window mask on last P cols: keep tk >= tq_local-512
                nc.gpsimd.affine_select(
                    out=at2[:, :, WIN : WIN + P], in_=at2[:, :, WIN : WIN + P],
                    compare_op=GE, fill=0.0,
                    base=0, pattern=[[0, 2], [-1, P]], channel_multiplier=1)

            if j % 4 == 3:
                do_quad(j // 4)

        aps_cm.__exit__(None, None, None)
        ap_cm.__exit__(None, None, None)

        # =================== tail: out projection ===================
        with tc.tile_pool(name="tpsum", bufs=1, space="PSUM") as tps:
            dtr = tps.tile([P, 2 * NT], F32, tag="dtr", bufs=1, name="dtr")
            nc.tensor.transpose(dtr[:], den_rows[:], ident32[:])
            nc.vector.reciprocal(
                rden[:].rearrange("p h (q n) -> p h q n", n=4),
                dtr[:].rearrange("p (q h n) -> p h q n", h=2, n=4))
            for n in range(NT):
                o_ps = tps.tile([P, DM], F32, tag="op0", bufs=3, name=f"op0_{n}")
                for c in range(2):
                    nc.tensor.matmul(o_ps[:], attoT[:, 0, c, ts(n, P)],
                                     wo_sb[:, c, :], start=(c == 0), stop=(c == 1))
                nc.scalar.activation(o_sb[:, n, :], o_ps[:], COPY,
                                     scale=rden[:, 0, n : n + 1])
                o_ps2 = tps.tile([P, DM], F32, tag="op1", bufs=3, name=f"op1_{n}")
                for c in range(2):
                    nc.tensor.matmul(o_ps2[:], attoT[:, 1, c, ts(n, P)],
                                     wo_sb[:, 2 + c, :], start=(c == 0), stop=(c == 1))
                nc.vector.scalar_tensor_tensor(
                    out=o_sb[:, n, :], in0=o_ps2[:], scalar=rden[:, 1, n : n + 1],
                    in1=o_sb[:, n, :], op0=MUL, op1=ADD)
                if n % 2 == 1:
                    nc.sync.dma_start(out=o_d[:, n - 1 : n + 1, :],
                                      in_=o_sb[:, n - 1 : n + 1, :])
    nc.compile()
    return nc


# ======================= host side =======================

_PROGRAMS = {}


def _get_program(apply_wprod: bool):
    key = bool(apply_wprod)
    if key not in _PROGRAMS:
        _PROGRAMS[key] = build_program(key)
    return _PROGRAMS[key]


_DEINT = np.concatenate([np.arange(0, DH, 2), np.arange(1, DH, 2)])


def _rope_tables():
    freqs = ROPE_BASE ** (-2.0 * np.arange(DH // 2, dtype=np.float64) / DH)
    theta = np.arange(T, dtype=np.float64)[None, :] * freqs[:, None]
    return np.cos(theta), np.sin(theta)


def _prep_inputs(x, wq, wkv, wo, q_norm_w, k_norm_w):
    x2 = np.asarray(x, dtype=np.float32).reshape(T, DM)
    wq = np.asarray(wq, dtype=np.float32)
    wkv = np.asarray(wkv, dtype=np.float32)
    wo = np.asarray(wo, dtype=np.float32)
    wk = wkv[:DH]
    wv = wkv[DH:]
    cos, sin = _rope_tables()

    # x transposed: xh[p, cc, t] = x[t, 128cc+p]
    xh = np.ascontiguousarray(
        x2.T.reshape(2, P, T).transpose(1, 0, 2).astype(np.float16))
    # cs[p, row, t] rows [cos, sin, -sin, cos]
    csh = np.ascontiguousarray(
        np.stack([cos, sin, -sin, cos], axis=1).astype(np.float16))
    # k/v: [p, cc, dh_out]
    wkT = wk[_DEINT].T.astype(np.float16)          # [dm, dh]
    wvT = wv.T.astype(np.float16)
    wkh = np.ascontiguousarray(wkT.reshape(2, P, DH).transpose(1, 0, 2))
    wvh = np.ascontiguousarray(wvT.reshape(2, P, DH).transpose(1, 0, 2))

    wprod = (np.asarray(q_norm_w, np.float32) * np.asarray(k_norm_w, np.float32))
    apply_wprod = not np.allclose(wprod, 1.0)
    wprod_de = wprod[_DEINT]
    wprod_cols = np.ascontiguousarray(
        np.stack([wprod_de[:P], wprod_de[P:]], axis=1).astype(np.float32))

    in_maps = []
    for c in range(NCORES):
        heads = [HPC * c + i for i in range(HPC)]
        wqh = np.empty((P, 2, 2, DH), dtype=np.float16)   # [p, cc, h, dh]
        for hl, h in enumerate(heads):
            wqT = wq[DH * h : DH * (h + 1)][_DEINT].T.astype(np.float16)  # [dm, dh]
            wqh[:, :, hl, :] = wqT.reshape(2, P, DH).transpose(1, 0, 2)
        woh = np.empty((P, 4, DM), dtype=np.float16)      # [p, 2h+c, dm]
        for hl, h in enumerate(heads):
            woT = wo[:, DH * h : DH * (h + 1)].T.astype(np.float16)  # [dh, dm]
            woh[:, 2 * hl, :] = woT[0:P]
            woh[:, 2 * hl + 1, :] = woT[P : 2 * P]
        m = {
            "xh": xh, "wqh": np.ascontiguousarray(wqh), "wkh": wkh, "wvh": wvh,
            "woh": np.ascontiguousarray(woh), "csh": csh,
        }
        if apply_wprod:
            m["wprod"] = wprod_cols
        in_maps.append(m)
    return in_maps, apply_wprod


def _run(inputs, trace=False, trace_kwargs=None):
    from concourse.bass_utils import run_bass_kernel_spmd

    in_maps, apply_wprod = _prep_inputs(**inputs)
    nc = _get_program(apply_wprod)
    res = run_bass_kernel_spmd(
        nc, in_maps, list(range(NCORES)), trace=trace, **(trace_kwargs or {})
    )
    out = np.zeros((T, DM), dtype=np.float32)
    for c in range(NCORES):
        oc = np.asarray(res.results[c]["o"], dtype=np.float32)  # [p, n, dm]
        out += oc.transpose(1, 0, 2).reshape(T, DM)
    return out.reshape(1, T, DM), res


def kernel(**inputs):
    out, _ = _run(inputs, trace=False)
    return out


# revision 30
# speedup vs baseline: 1.0651x; 1.0144x over previous
"""Trainium2 Bass kernel v3 for nn_Attention_70248485093458 (sliding-window attention).

8 cores x 2 query heads, 1 shared KV head. fp16 data paths, f32 psum/stats.

v3 changes over v2:
  - input DMA reorder (wv/wq-h0 first) + PE warmup dummies -> HAM warm early
  - host sends 4 rope rows [cos,sin,-sin,cos] (no device derivation)
  - one merged ACT staging copy per tile (v,k,q0,q1), sqs reads fp16 staging
  - both heads share one EXP / one affine_select per k-tile (merged psum)
  - all rs-chain SQRTs precede the first EXP -> 2 ACT table loads total
  - den psum DMA'd directly to DRAM; per-quad den row prefetch for the tail
  - attention A-parts j=0..6 moved after rs_block(3) at end of front
"""

import numpy as np

import concourse.bass as bass
import concourse.mybir as mybir
import concourse.tile as tile
from concourse import bacc
from concourse.bass import ts, ds
from concourse.masks import make_identity

T = 2048
DM = 256
DH = 256
WIN = 512
NCORES = 8
HPC = 2
P = 128
NT = 16
NB = 4
NQ = 4           # tq quads of 512
EPS = 1.1920929e-07
ROPE_BASE = 10000.0

F32 = mybir.dt.float32
F16 = mybir.dt.float16
F8 = mybir.dt.float8e4
DR = mybir.MatmulPerfMode.DoubleRow
MUL = mybir.AluOpType.mult
ADD = mybir.AluOpType.add
SUB = mybir.AluOpType.subtract
GE = mybir.AluOpType.is_ge
EXP = mybir.ActivationFunctionType.Exp
SQRT = mybir.ActivationFunctionType.Sqrt
COPY = mybir.ActivationFunctionType.Copy
SQUARE = mybir.ActivationFunctionType.Square


def _band(j):
    return min(WIN + P, T - P * j)


def build_program(apply_wprod: bool):
    nc = bacc.Bacc(None, target_bir_lowering=False)
    with tile.TileContext(nc) as tc, \
         tc.tile_pool(name="dram", bufs=1, space="DRAM") as dram, \
         tc.tile_pool(name="pp", bufs=1) as pp:
        x_d = dram.tile([P, 2, T], F16, kind="ExternalInput", name="xh", uniquify=False)
        wq_d = dram.tile([P, 2, 2, DH], F16, kind="ExternalInput", name="wqh", uniquify=False)
        wk_d = dram.tile([P, 2, DH], F16, kind="ExternalInput", name="wkh", uniquify=False)
        wv_d = dram.tile([P, 2, DH], F16, kind="ExternalInput", name="wvh", uniquify=False)
        wo_d = dram.tile([P, 4, DM], F16, kind="ExternalInput", name="woh", uniquify=False)
        # rope tables: host sends all 4 rows [cos, sin, -sin, cos]
        cs_d = dram.tile([P, 4, T], F16, kind="ExternalInput", name="csh", uniquify=False)
        wprod_d = None
        if apply_wprod:
            wprod_d = dram.tile([P, 2], F32, kind="ExternalInput", name="wprod", uniquify=False)
        o_d = dram.tile([P, NT, DM], F16, kind="ExternalOutput", name="o", uniquify=False)
        den_d = dram.tile([NQ, 2, 512], F32, kind="Internal", name="denq")

        # ---- persistent SBUF ----
        x_sb = pp.tile([P, 2, T], F16)
        wq_sb = pp.tile([P, 2, 2, DH], F16)
        wk_sb = pp.tile([P, 2, DH], F16)
        wv_sb = pp.tile([P, 2, DH], F16)
        wo_sb = pp.tile([P, 4, DM], F16)
        cs_sb = pp.tile([P, 4, T], F16)
        # staging: [.., 0, :]=v  [.., 1, :]=k  [.., 2+h, :]=q_h
        st_sb = pp.tile([P, NT, 4, DH], F16)
        # roped, normalized, transposed q/k: [p, head, comp, t] / [p, comp, t]
        # fp8e4: scores run as single DoubleRow matmuls (d=256 contraction)
        qTr = pp.tile([P, 2, 2, T], F8)
        kTr = pp.tile([P, 2, T], F8)
        # ssq -> rs in place: rows q0, q1, k
        ss = pp.tile([P, 3, NT], F32)
        attoT = pp.tile([P, 2, 2, T], F16)
        den_sb = pp.tile([1, 2, T], F32)
        rden = pp.tile([P, 2, NT], F32)
        den_rows = pp.tile([2 * NT, P], F32)   # rows (q, h, u)
        o_sb = pp.tile([P, NT, DM], F16)
        ones_h = pp.tile([P, 1], F16)
        ident = pp.tile([P, P], F16)
        ident32 = pp.tile([2 * NT, 2 * NT], F32)
        wprod_sb = pp.tile([P, 2], F32) if apply_wprod else None

        atp = pp.tile([P, 2, 7, 640], F16)
        negb = pp.tile([P, 1], F32)
        wscr = pp.tile([P, 512], F16)
        sqscr = pp.tile([P, 1], F32)
        nc.vector.memset(ones_h[:], 1.0)
        # exp offset: |s·rs_k| <= 16 hard (Cauchy-Schwarz on unit-rms vectors), so
        # exp(s-7) <= e^9 = 8103 fits fp16; typical s~N(0,1) keeps weights normal.
        nc.vector.memset(negb[:], -7.0)
        nc.vector.memset(wscr[:], 0.0)
        nc.vector.memset(sqscr[:], 1.0)
        # preload the sqrt ACT table during the input-DMA window; everything up
        # to the first EXP (copies, squares, sqrts) lives in the sqrt set.
        nc.scalar.activation(sqscr[:], sqscr[:], SQRT)
        make_identity(nc, ident[:])
        make_identity(nc, ident32[:])

        # ---- input DMAs ---- (minimal first-tile set leads each queue)
        nc.sync.dma_start(out=x_sb[:, :, 0:256], in_=x_d[:, :, 0:256])
        nc.sync.dma_start(out=x_sb[:, :, 256:512], in_=x_d[:, :, 256:512])
        nc.scalar.dma_start(out=wv_sb[:], in_=wv_d[:])
        nc.scalar.dma_start(out=wq_sb[:, :, 0, :], in_=wq_d[:, :, 0, :])
        nc.sync.dma_start(out=wq_sb[:, :, 1, :], in_=wq_d[:, :, 1, :])
        nc.scalar.dma_start(out=wk_sb[:], in_=wk_d[:])
        for g in range(1, 4):
            nc.sync.dma_start(out=x_sb[:, :, ts(g, 512)], in_=x_d[:, :, ts(g, 512)])
        nc.scalar.dma_start(out=cs_sb[:, :, 0:512], in_=cs_d[:, :, 0:512])
        for g in range(1, 4):
            nc.scalar.dma_start(out=cs_sb[:, :, ts(g, 512)], in_=cs_d[:, :, ts(g, 512)])
        nc.sync.dma_start(out=wo_sb[:], in_=wo_d[:])
        if apply_wprod:
            nc.sync.dma_start(out=wprod_sb[:], in_=wprod_d[:])

        # =================== front ===================
        fp_cm = tc.tile_pool(name="fwork", bufs=1)
        fps_cm = tc.tile_pool(name="fpsum", bufs=1, space="PSUM")
        fw = fp_cm.__enter__()
        fps = fps_cm.__enter__()

        # PE warmup: ~12 dummy matmuls so HAM un-throttles before projections.
        # Shares a pp slot (released before the second projection needs it).
        warm_ps = fps.tile([1, 512], F32, tag="pp", bufs=2, name="warm")
        for _ in range(12):
            nc.tensor.matmul(warm_ps[:], ones_h[:], wscr[:], start=True, stop=True)

        attn = {}
        for _h in range(2):
            for _j in range(7):
                attn[(_h, _j)] = atp[:, _h, _j, :]
        pp_t = {}   # per-tile proj psum

        def proj_tile(n):
            # layout: v@bank0, k@bank0+1KB, q0@bank1, q1@bank1+1KB. Open
            # accumulation groups must not interleave within a bank, so pair
            # (v,q0) then (k,q1) — each pair hits two different banks.
            pps = fps.tile([P, 4, DH], F32, tag="pp", bufs=2, name=f"pp{n}")
            pp_t[n] = pps
            for cc in range(2):
                st = x_sb[:, cc, ts(n, P)]
                nc.tensor.matmul(pps[:, 0, :], st, wv_sb[:, cc, :],
                                 start=(cc == 0), stop=(cc == 1))
                nc.tensor.matmul(pps[:, 2, :], st, wq_sb[:, cc, 0, :],
                                 start=(cc == 0), stop=(cc == 1))
            for cc in range(2):
                st = x_sb[:, cc, ts(n, P)]
                nc.tensor.matmul(pps[:, 1, :], st, wk_sb[:, cc, :],
                                 start=(cc == 0), stop=(cc == 1))
                nc.tensor.matmul(pps[:, 3, :], st, wq_sb[:, cc, 1, :],
                                 start=(cc == 0), stop=(cc == 1))

        def ssq_copy_tile(n):
            pps = pp_t[n]
            # one merged staging copy: [v|k|q0|q1] psum -> st_sb fp16
            nc.scalar.activation(st_sb[:, n, :, :], pps[:], COPY)
            # ssq q1 on DVE (STT square + accum); q0/k on Scalar — balance
            sq0 = fw.tile([P, DH], F16, tag="sq0", bufs=4, name=f"sq0_{n}")
            nc.vector.scalar_tensor_tensor(
                out=sq0[:], in0=st_sb[:, n, 3, :], scalar=1.0,
                in1=st_sb[:, n, 3, :],
                op0=MUL, op1=MUL, accum_out=ss[:, 1, n : n + 1],
            )
            sqs = fw.tile([P, DH], F16, tag="sqs", bufs=4, name=f"sqs{n}")
            nc.scalar.activation(sqs[:], st_sb[:, n, 2, :], SQUARE,
                                 accum_out=ss[:, 0, n : n + 1])
            nc.scalar.activation(sqs[:], st_sb[:, n, 1, :], SQUARE,
                                 accum_out=ss[:, 2, n : n + 1])

        def rs_block(b):
            # one merged chain: rows q0,q1,k all use 1/sqrt(ssq + DH*EPS)
            # = 1/(16*rms).  q rows carry the extra 1/16 = score scale; k row
            # gets plain 1/rms (multiply by 16).
            cs3 = (slice(None), slice(0, 3), ds(4 * b, 4))
            nc.vector.tensor_scalar(out=ss[cs3], in0=ss[cs3],
                                    scalar1=1.0, scalar2=DH * EPS, op0=MUL, op1=ADD)
            nc.scalar.activation(ss[cs3], ss[cs3], SQRT)
            nc.vector.reciprocal(ss[cs3], ss[cs3])
            nc.vector.tensor_scalar(out=ss[:, 2, ds(4 * b, 4)],
                                    in0=ss[:, 2, ds(4 * b, 4)],
                                    scalar1=16.0, scalar2=0.0, op0=MUL, op1=ADD)

        def scale_tile(n):
            for h in range(2):
                nc.vector.tensor_scalar(out=st_sb[:, n, 2 + h, :],
                                        in0=st_sb[:, n, 2 + h, :],
                                        scalar1=ss[:, h, n : n + 1], scalar2=0.0,
                                        op0=MUL, op1=ADD)

        tr_b = {}

        def transpose_block(b):
            # per-tensor psum tiles (1 bank each) so transposes of the next
            # tensor can start while rope still reads the previous one
            for t3 in range(3):
                trp = fps.tile([P, 2, 4, P], F16, tag="tr", bufs=4,
                               name=f"tr{b}_{t3}")
                tr_b[(b, t3)] = trp
                for u in range(4):
                    n = 4 * b + u
                    src = st_sb[:, n, 1, :] if t3 == 2 else st_sb[:, n, 2 + t3, :]
                    for c in range(2):
                        nc.tensor.transpose(
                            trp[:, c, u, :], src[:, ts(c, P)], ident[:]
                        )

        def rope_block(b):
            # merged-mul rope: u_e = [e|e] * [cos|sin], u_o = [o|o] * [-sin|cos],
            # [r_e|r_o] = u_e + u_o.  muls DVE (psum reads), add split Pool/DVE.
            sp = ts(b, 512)
            for t3 in range(3):
                trp = tr_b[(b, t3)]
                if t3 == 2:
                    outro = kTr[:, :, sp]
                else:
                    outro = qTr[:, t3, :, sp]
                # one [P,2,2,512] mul: (te,to) x ([cos,sin],[-sin,cos])
                u4 = fw.tile([P, 2, 2, 512], F16, tag=f"u{t3}", bufs=2,
                             name=f"u4_{t3}{b}")
                t2 = trp[:].rearrange("p a b c -> p a (b c)") \
                    .rearrange("p a (x d) -> p a x d", x=1) \
                    .broadcast_to((P, 2, 2, 512))
                c2 = cs_sb[:, :, sp].rearrange("p (a b) t -> p a b t", a=2)
                nc.vector.tensor_mul(u4[:], t2, c2)
                # split the add across Pool (e) and DVE (o) — Pool is ~2x slower
                nc.gpsimd.tensor_add(outro[:, 0, :], u4[:, 0, 0, :], u4[:, 1, 0, :])
                nc.vector.tensor_add(outro[:, 1, :], u4[:, 0, 1, :], u4[:, 1, 1, :])
            if apply_wprod:
                for c in range(2):
                    nc.vector.tensor_scalar(
                        out=kTr[:, c, sp], in0=kTr[:, c, sp],
                        scalar1=wprod_sb[:, c : c + 1], scalar2=0.0, op0=MUL, op1=ADD)

        def a_part(pj):
            sAp = fps.tile([P, 2, 512], F32, tag="pp", bufs=2, name=f"sAp{pj}")
            for ph in range(2):
                nc.tensor.matmul(
                    sAp[:, ph, :], kTr[:, :, ts(pj, P)],
                    qTr[:, ph, :, ds(P * pj, 512)],
                    start=True, stop=True, perf_mode=DR)
            nc.scalar.activation(atp[:, :, pj, 0:512], sAp[:], EXP,
                                 scale=ss[:, 2, pj : pj + 1], bias=negb[:])
            nc.gpsimd.affine_select(
                out=atp[:, :, pj, 0:P], in_=atp[:, :, pj, 0:P], compare_op=GE,
                fill=0.0, base=0, pattern=[[0, 2], [1, P]],
                channel_multiplier=-1)

        # front schedule: transposes of block b-1 ride behind projections of b;
        # attention A-parts interleave once their qTr blocks (j+4)//4 are roped.
        for b in range(NB):
            for u in range(4):
                proj_tile(4 * b + u)
                ssq_copy_tile(4 * b + u)
            rs_block(b)
            for u in range(4):
                scale_tile(4 * b + u)
            if b >= 1:
                transpose_block(b - 1)
                rope_block(b - 1)
            if b == 3:
                # all rs_block SQRTs precede the first EXP (2 ACT table loads)
                for pj in range(7):
                    a_part(pj)
        transpose_block(NB - 1)
        # second warm burst: fills the PE-idle window while DVE ropes the last
        # block, so attention opens at full clock (HAM un-throttled)
        warm2 = fps.tile([1, 512], F32, tag="pp", bufs=2, name="warm2")
        for _ in range(14):
            nc.tensor.matmul(warm2[:], ones_h[:], wscr[:], start=True, stop=True)
        rope_block(NB - 1)

        fps_cm.__exit__(None, None, None)
        fp_cm.__exit__(None, None, None)

        # =================== attention ===================
        ap_cm = tc.tile_pool(name="awork", bufs=1)
        aps_cm = tc.tile_pool(name="apsum", bufs=1, space="PSUM")
        aw = ap_cm.__enter__()
        aps = aps_cm.__enter__()

        def do_quad(q):
            js = list(range(max(0, 4 * q - 4), min(NT, 4 * q + 4)))
            js.remove(4 * q)
            js.insert(0, 4 * q)  # full-width segment first (uniform has_written)
            # both heads interleaved per segment: each v stationary loads once
            a2 = [aps.tile([P, 2, 512], F32, tag="pv", bufs=2, name=f"aq{h}_{q}")
                  for h in range(2)]
            for ji, jj in enumerate(js):
                wj = _band(jj)
                lo = max(0, 512 * q - P * jj)
                hi = min(wj, 512 * q + 512 - P * jj)
                po = P * jj + lo - 512 * q
                for c in range(2):
                    for h in range(2):
                        nc.tensor.matmul(
                            a2[h][:, c, po : po + hi - lo],
                            st_sb[:, jj, 0, ts(c, P)], attn[(h, jj)][:, lo:hi],
                            start=(ji == 0), stop=(ji == len(js) - 1))
            for h in range(2):
                d_ps = aps.tile([1, 512], F32, tag="dq", bufs=1, name=f"dq{h}_{q}")
                for ji, jj in enumerate(js):
                    wj = _band(jj)
                    lo = max(0, 512 * q - P * jj)
                    hi = min(wj, 512 * q + 512 - P * jj)
                    po = P * jj + lo - 512 * q
                    nc.tensor.matmul(
                        d_ps[:, po : po + hi - lo], ones_h[:],
                        attn[(h, jj)][:, lo:hi],
                        start=(ji == 0), stop=(ji == len(js) - 1))
                nc.vector.tensor_copy(attoT[:, h, :, ts(q, 512)], a2[h][:])
                nc.scalar.activation(den_sb[:, h, ts(q, 512)], d_ps[:], COPY)
                # per-quad den bounce to DRAM while the sync queue is idle
                nc.sync.dma_start(out=den_d[q, h, :],
                                  in_=den_sb[:, h, ts(q, 512)])
                # prefetch den rows for the tail transpose
                nc.sync.dma_start(
                    out=den_rows[ds(8 * q + 4 * h, 4), :],
                    in_=den_d[q, h, :].rearrange("(n u) -> n u", u=P))

        for j in range(NT):
            w = _band(j)
            wA = min(w, 512)
            if j < 7:
                # A-parts were computed during the front phase; finish B-parts
                sBp = aps.tile([P, 2, P], F32, tag="sB", bufs=1, name=f"sBp_{j}")
                for h in range(2):
                    nc.tensor.matmul(
                        sBp[:, h, :], kTr[:, :, ts(j, P)],
                        qTr[:, h, :, ds(P * j + 512, P)],
                        start=True, stop=True, perf_mode=DR)
                nc.scalar.activation(atp[:, :, j, 512:640], sBp[:], EXP,
                                     scale=ss[:, 2, j : j + 1], bias=negb[:])
                nc.gpsimd.affine_select(
                    out=atp[:, :, j, WIN : WIN + P], in_=atp[:, :, j, WIN : WIN + P],
                    compare_op=GE, fill=0.0,
                    base=0, pattern=[[0, 2], [-1, P]], channel_multiplier=1)
                if j % 4 == 3:
                    do_quad(j // 4)
                continue
            # heads share one psum/EXP; per c each kTr stationary loads once
            at2 = aw.tile([P, 2, 640], F16, tag="at", bufs=9, name=f"at_{j}")
            for h in range(2):
                attn[(h, j)] = at2[:, h, :]
            sB = (aps.tile([P, 2, P], F32, tag="sB", bufs=1, name=f"sB_{j}")
                  if w == 640 else None)
            for h in range(2):
                sAh = aps.tile([P, 512], F32, tag="sA", bufs=2, name=f"sA{h}_{j}")
                nc.tensor.matmul(
                    sAh[:, 0:wA], kTr[:, :, ts(j, P)],
                    qTr[:, h, :, ds(P * j, wA)],
                    start=True, stop=True, perf_mode=DR)
                if w == 640:
                    nc.tensor.matmul(
                        sB[:, h, :], kTr[:, :, ts(j, P)],
                        qTr[:, h, :, ds(P * j + 512, P)],
                        start=True, stop=True, perf_mode=DR)
                nc.scalar.activation(at2[:, h, 0:wA], sAh[:, 0:wA], EXP,
                                     scale=ss[:, 2, j : j + 1], bias=negb[:])
            if w == 640:
                nc.scalar.activation(at2[:, :, 512:640], sB[:], EXP,
                                     scale=ss[:, 2, j : j + 1], bias=negb[:])
            # causal mask on diagonal P cols: keep tq_local >= tk
            nc.gpsimd.affine_select(
                out=at2[:, :, 0:P], in_=at2[:, :, 0:P], compare_op=GE, fill=0.0,
                base=0, pattern=[[0, 2], [1, P]], channel_multiplier=-1)
            if w == 640:
                # window mask on last P cols: keep tk >= tq_local-512
                nc.gpsimd.affine_select(
                    out=at2[:, :, WIN : WIN + P], in_=at2[:, :, WIN : WIN + P],
                    compare_op=GE, fill=0.0,
                    base=0, pattern=[[0, 2], [-1, P]], channel_multiplier=1)

            if j % 4 == 3:
                do_quad(j // 4)

        aps_cm.__exit__(None, None, None)
        ap_cm.__exit__(None, None, None)

        # =================== tail: out projection ===================
        with tc.tile_pool(name="tpsum", bufs=1, space="PSUM") as tps:
            dtr = tps.tile([P, 2 * NT], F32, tag="dtr", bufs=1, name="dtr")
            nc.tensor.transpose(dtr[:], den_rows[:], ident32[:])
            nc.vector.reciprocal(
                rden[:].rearrange("p h (q n) -> p h q n", n=4),
                dtr[:].rearrange("p (q h n) -> p h q n", h=2, n=4))
            for n in range(NT):
                o_ps = tps.tile([P, DM], F32, tag="op0", bufs=3, name=f"op0_{n}")
                for c in range(2):
                    nc.tensor.matmul(o_ps[:], attoT[:, 0, c, ts(n, P)],
                                     wo_sb[:, c, :], start=(c == 0), stop=(c == 1))
                nc.scalar.activation(o_sb[:, n, :], o_ps[:], COPY,
                                     scale=rden[:, 0, n : n + 1])
                o_ps2 = tps.tile([P, DM], F32, tag="op1", bufs=3, name=f"op1_{n}")
                for c in range(2):
                    nc.tensor.matmul(o_ps2[:], attoT[:, 1, c, ts(n, P)],
                                     wo_sb[:, 2 + c, :], start=(c == 0), stop=(c == 1))
                nc.vector.scalar_tensor_tensor(
                    out=o_sb[:, n, :], in0=o_ps2[:], scalar=rden[:, 1, n : n + 1],
                    in1=o_sb[:, n, :], op0=MUL, op1=ADD)
                if n % 2 == 1:
                    nc.sync.dma_start(out=o_d[:, n - 1 : n + 1, :],
                                      in_=o_sb[:, n - 1 : n + 1, :])
    nc.compile()
    return nc


# ======================= host side =======================

_PROGRAMS = {}


def _get_program(apply_wprod: bool):
    key = bool(apply_wprod)
    if key not in _PROGRAMS:
        _PROGRAMS[key] = build_program(key)
    return _PROGRAMS[key]


_DEINT = np.concatenate([np.arange(0, DH, 2), np.arange(1, DH, 2)])


def _rope_tables():
    freqs = ROPE_BASE ** (-2.0 * np.arange(DH // 2, dtype=np.float64) / DH)
    theta = np.arange(T, dtype=np.float64)[None, :] * freqs[:, None]
    return np.cos(theta), np.sin(theta)


def _prep_inputs(x, wq, wkv, wo, q_norm_w, k_norm_w):
    x2 = np.asarray(x, dtype=np.float32).reshape(T, DM)
    wq = np.asarray(wq, dtype=np.float32)
    wkv = np.asarray(wkv, dtype=np.float32)
    wo = np.asarray(wo, dtype=np.float32)
    wk = wkv[:DH]
    wv = wkv[DH:]
    cos, sin = _rope_tables()

    # x transposed: xh[p, cc, t] = x[t, 128cc+p]
    xh = np.ascontiguousarray(
        x2.T.reshape(2, P, T).transpose(1, 0, 2).astype(np.float16))
    # cs[p, row, t] rows [cos, sin, -sin, cos]
    csh = np.ascontiguousarray(
        np.stack([cos, sin, -sin, cos], axis=1).astype(np.float16))
    # k/v: [p, cc, dh_out]
    wkT = wk[_DEINT].T.astype(np.float16)          # [dm, dh]
    wvT = wv.T.astype(np.float16)
    wkh = np.ascontiguousarray(wkT.reshape(2, P, DH).transpose(1, 0, 2))
    wvh = np.ascontiguousarray(wvT.reshape(2, P, DH).transpose(1, 0, 2))

    wprod = (np.asarray(q_norm_w, np.float32) * np.asarray(k_norm_w, np.float32))
    apply_wprod = not np.allclose(wprod, 1.0)
    wprod_de = wprod[_DEINT]
    wprod_cols = np.ascontiguousarray(
        np.stack([wprod_de[:P], wprod_de[P:]], axis=1).astype(np.float32))

    in_maps = []
    for c in range(NCORES):
        heads = [HPC * c + i for i in range(HPC)]
        wqh = np.empty((P, 2, 2, DH), dtype=np.float16)   # [p, cc, h, dh]
        for hl, h in enumerate(heads):
            wqT = wq[DH * h : DH * (h + 1)][_DEINT].T.astype(np.float16)  # [dm, dh]
            wqh[:, :, hl, :] = wqT.reshape(2, P, DH).transpose(1, 0, 2)
        woh = np.empty((P, 4, DM), dtype=np.float16)      # [p, 2h+c, dm]
        for hl, h in enumerate(heads):
            woT = wo[:, DH * h : DH * (h + 1)].T.astype(np.float16)  # [dh, dm]
            woh[:, 2 * hl, :] = woT[0:P]
            woh[:, 2 * hl + 1, :] = woT[P : 2 * P]
        m = {
            "xh": xh, "wqh": np.ascontiguousarray(wqh), "wkh": wkh, "wvh": wvh,
            "woh": np.ascontiguousarray(woh), "csh": csh,
        }
        if apply_wprod:
            m["wprod"] = wprod_cols
        in_maps.append(m)
    return in_maps, apply_wprod


def _run(inputs, trace=False, trace_kwargs=None):
    from concourse.bass_utils import run_bass_kernel_spmd

    in_maps, apply_wprod = _prep_inputs(**inputs)
    nc = _get_program(apply_wprod)
    res = run_bass_kernel_spmd(
        nc, in_maps, list(range(NCORES)), trace=trace, **(trace_kwargs or {})
    )
    out = np.zeros((T, DM), dtype=np.float32)
    for c in range(NCORES):
        oc = np.asarray(res.results[c]["o"], dtype=np.float32)  # [p, n, dm]
        out += oc.transpose(1, 0, 2).reshape(T, DM)
    return out.reshape(1, T, DM), res


def kernel(**inputs):
    out, _ = _run(inputs, trace=False)
    return out
